# revision 27
# baseline (speedup 1.0000x reference)
"""DDiT block kernel for 8 Trainium2 NeuronCores — v3 (pipelined attention).

Sharding: core c = (b = c//2, half = c%2).  Each core computes the block
output for its (batch, sequence-half): 1024 rows of 2048.  K/V are computed
redundantly for the full sequence on each core.  No collectives.

v3 changes vs v2 (trace-driven):
- Attention rebuilt as a q-chunked (512) software pipeline with
  double-buffered score PSUM: ScalarE streams one Exp per k-tile
  ([128,1024] over both heads of a pair) back-to-back while the PE runs
  2 k-tiles ahead on scores and 1 behind on attn@V.  v2 serialised
  PE->exp->PE per k-tile (bufs=1 score psum), which also kept the PE
  HAM-throttled at half clock for the whole phase (~620us -> target ~300).
- wqkv DMA split per d-tile so the first QKV matmul starts after ~1/8 of
  the weight load; weight streams (wqkv/wout/w2/w1) moved to the second
  HWDGE queue (ScalarE-issued) so they never queue behind x loads and
  SBUF transposes on the sync queue.
- fc1 weights re-laid-out on host to [jt, p, dc*128] so each per-jt DMA
  reads 2KB contiguous per partition (v2's layout produced 256B packets
  at ~74GB/s, gating fc1).
"""

import os
from contextlib import ExitStack

import numpy as np

B, S, D, H = 4, 2048, 1024, 16
HD = D // H  # 64
J = 4 * D  # 4096
SO = S // 2  # 1024 rows per core
SF = S  # full sequence
P = 128
EPS = 1e-6
N_CORES = 8

ST_F = SF // P  # 16 s-tiles full seq
ST_O = SO // P  # 8 s-tiles own half
DC = D // P  # 8 d-tiles
JT = J // P  # 32 j-tiles
HP = H // 2  # 8 head pairs


def _emit(ctx, nc, tc, io):
    import concourse.bass as bass
    import concourse.mybir as mybir

    f32 = mybir.dt.float32
    bf16 = mybir.dt.bfloat16
    f16 = mybir.dt.float16
    f8 = mybir.dt.float8e4
    AF = mybir.ActivationFunctionType
    OP = mybir.AluOpType
    DR = mybir.MatmulPerfMode.DoubleRow

    def big(shape, dtype, name, side="left"):
        t, free = tc.tile(shape, dtype, name=name, side=side)
        return t, free

    def bcast_row(pool, key, n=D, dt=f32, engine=None):
        """DMA a [n] dram row into a [P, n] sbuf tile, replicated across partitions."""
        ap_1d = io[key].ap()
        t = pool.tile([P, n], dt, tag=f"row_{key}", name=f"row_{key}", bufs=1)
        src = bass.AP(
            tensor=ap_1d.tensor,
            offset=ap_1d.offset,
            ap=[[0, P], list(ap_1d.ap[0])],
        )
        (engine or nc.sync).dma_start(out=t, in_=src)
        return t

    NB = 512  # psum bank width (f32 cols); matmuls may not cross banks

    def mmw(ps, lhsT, rhs, start, stop):
        for o in range(0, rhs.shape[-1], NB):
            nc.tensor.matmul(
                ps[:, o : o + NB], lhsT, rhs[..., o : o + NB],
                start=start, stop=stop,
            )

    eps_t, _free_eps = tc.tile([P, 1], f32, name="eps_t")
    nc.vector.memset(eps_t, EPS)

    def layernorm_mod(pool, x_ap, out_bf, a_bf, c_bf):
        # out_bf16 = ((x - mean) * rstd) * A + C   (A, C bf16 rows)
        stats = pool.tile([P, 2, 6], f32, tag="bnstats", name="stats")
        mv = pool.tile([P, 2], f32, tag="bnaggr", name="mv")
        xv = x_ap.rearrange("p (g k) -> p g k", g=2)
        for g in range(2):
            nc.vector.bn_stats(out=stats[:, g, :], in_=xv[:, g, :])
        nc.vector.bn_aggr(out=mv, in_=stats)
        rstd = pool.tile([P, 1], f32, tag="rstd", name="rstd")
        nc.scalar.activation(out=rstd, in_=mv[:, 1:2], func=AF.Sqrt, bias=eps_t)
        nc.vector.reciprocal(out=rstd, in_=rstd)
        tmp = pool.tile([P, D], bf16, tag="lntmp", name="lntmp")
        nc.vector.tensor_scalar(
            out=tmp, in0=x_ap, scalar1=mv[:, 0:1], scalar2=rstd,
            op0=OP.subtract, op1=OP.mult,
        )
        tmp2 = pool.tile([P, D], bf16, tag="lntmp2", name="lntmp2")
        nc.vector.tensor_mul(tmp2, tmp, a_bf)
        nc.vector.tensor_add(out_bf, tmp2, c_bf)

    # ======== persistent SBUF tensors (phase A) ========
    wqkv_sb, free_wqkv = big([P, DC, 3 * D], bf16, "wqkv_sb")  # 48KB/p
    yT, free_yT = big([P, DC, 2 * P], bf16, "yT")  # rolling 2-tile buffer
    qT, free_qT = big([P, HP, SO], bf16, "qT", side="right")
    kT, free_kT = big([P, HP, SF], bf16, "kT", side="right")
    v_aug, free_vaug = big([P, ST_F, H, 65], bf16, "v_aug", side="right")

    # ================ phase A: LN1 + QKV + rope, fused per s-tile ========
    with tc.tile_pool(name="pa", bufs=3) as pa, \
         tc.tile_pool(name="pas", bufs=4) as pas, \
         tc.tile_pool(name="psa", bufs=2, space="PSUM") as psa:
        # Warm the ACT sqrt/exp table set before the weight-DMA issue
        # instructions claim the ScalarE queue — the first LN rstd otherwise
        # waits ~14us for its PSEUDO_LOAD_ACT_FUNC_SET.
        warm_t = pa.tile([P, 1], f32, tag="warm", name="warm_t", bufs=1)
        nc.scalar.activation(out=warm_t, in_=eps_t, func=AF.Sqrt, bias=eps_t)
        # Small constants first on the scalar HWDGE queue; the wqkv chunk
        # issues are spread across the first loop iterations so the LN
        # rstd/modulate ACT work is never queued behind them.  The sync
        # queue starts with the x tiles so LN begins ASAP.
        a1_t = bcast_row(pa, "a1", dt=bf16, engine=nc.scalar)
        c1_t = bcast_row(pa, "c1", dt=bf16, engine=nc.scalar)
        cos_t = pa.tile([P, ST_F, HD], bf16, tag="cos", name="cos_t", bufs=1)
        sin_t = pa.tile([P, ST_F, HD], bf16, tag="sin", name="sin_t", bufs=1)
        # host pre-arranged to [p, t*d]: 2KB contiguous per partition
        nc.scalar.dma_start(cos_t, io["cos"].ap().rearrange("p (t d) -> p t d", d=HD))
        nc.scalar.dma_start(sin_t, io["sin"].ap().rearrange("p (t d) -> p t d", d=HD))
        wqkv_r = io["wqkv"].ap().rearrange("(dc p) c -> p dc c", p=P)
        wqkv_next = [0]

        def issue_wqkv(upto):
            while wqkv_next[0] < min(upto, DC):
                dc = wqkv_next[0]
                nc.scalar.dma_start(wqkv_sb[:, dc, :], wqkv_r[:, dc, :])
                wqkv_next[0] += 1

        issue_wqkv(2)
        nc.vector.memset(v_aug[:, :, :, 64:65], 1.0)

        def rope(out_bf, qn, st):
            """Rotary on bf16 [P, 1024] (16 head-halves of 64)."""
            qv = qn.rearrange("p (h d) -> p h d", h=16)
            cos_b = cos_t[:, st, None, :].to_broadcast((P, 16, HD))
            sin_b = sin_t[:, st, None, :].to_broadcast((P, 16, HD))
            t1 = pas.tile([P, 16, HD], bf16, tag="ropet1", name="t1")
            nc.vector.tensor_mul(t1, qv, cos_b)
            qshuf = qn.rearrange("p (h two j) -> p h two j", h=16, two=2)[:, :, ::-1, :]
            t2 = pas.tile([P, 16, HD], bf16, tag="ropet2", name="t2")
            nc.vector.tensor_mul(
                t2.rearrange("p h (two j) -> p h two j", two=2),
                qshuf,
                sin_b.rearrange("p h (two j) -> p h two j", two=2),
            )
            nc.vector.tensor_add(out_bf.rearrange("p (h d) -> p h d", h=16), t1, t2)

        for st in range(ST_F):
            src = io["x_own"] if st < ST_O else io["x_oth"]
            row0 = (st % ST_O) * P
            x_t = pa.tile([P, D], f32, tag="xin", name="x_t")
            nc.sync.dma_start(x_t, src.ap()[row0 : row0 + P, :])
            y_t = pa.tile([P, D], bf16, tag="y_nat", name="y_t")
            layernorm_mod(pas, x_t, y_t, a1_t, c1_t)
            # remaining wqkv chunks AFTER st0's LN ACT ops but BEFORE the
            # first matmuls that read them (reads must follow the DMA in
            # trace order or Tile cannot create the dependency)
            issue_wqkv(DC)
            sl = (st % 2) * P
            nc.sync.dma_start(
                out=yT[:, :, sl : sl + P], in_=y_t, transpose=True
            )

            blocks = []
            if st < ST_O:
                blocks.append(("q", 0))
            blocks.append(("k", D))
            blocks.append(("v", 2 * D))
            for kind, c0 in blocks:
                ps = psa.tile([P, D], f32, tag="qkv_ps", name="qkv_ps")
                for dc in range(DC):
                    mmw(
                        ps,
                        yT[:, dc, sl : sl + P],
                        wqkv_sb[:, dc, c0 : c0 + D],
                        start=(dc == 0),
                        stop=(dc == DC - 1),
                    )
                if kind == "v":
                    nc.scalar.copy(
                        out=v_aug[:, st, :, 0:64],
                        in_=ps.rearrange("p (h d) -> p h d", h=16),
                    )
                else:
                    qn = pa.tile([P, D], bf16, tag="qn", name="qn")
                    nc.scalar.copy(out=qn, in_=ps)
                    rot = pa.tile([P, D], bf16, tag="rot", name="rot")
                    rope(rot, qn, st)
                    dst = qT if kind == "q" else kT
                    nc.sync.dma_start(
                        out=dst[:, :, st * P : (st + 1) * P], in_=rot,
                        transpose=True,
                    )
    free_yT()
    free_wqkv()

    # ======== persistent SBUF tensors (phase B+) ========
    w2sb, free_w2 = big([P, JT, D], bf16, "w2sb")  # 64KB/p (in freed wqkv/yT space)
    attnT, free_attnT = big([P, DC, SO], bf16, "attnT")
    wout_sb, free_wout = big([P, DC, D], bf16, "wout_sb")

    # ======== phase B: attention (q-chunked, ScalarE-saturated pipeline) ====
    QW = 512  # q-chunk width; scores for both heads of a pair share one exp
    QH = SO // QW
    with tc.tile_pool(name="pb", bufs=4) as pb, \
         tc.tile_pool(name="pbs", bufs=2) as pbs, \
         tc.tile_pool(name="ps_sc", bufs=2, space="PSUM") as ps_sc, \
         tc.tile_pool(name="ps_u", bufs=2, space="PSUM") as ps_u:
        # prefetch wout + w2 (pre-scaled by gate_mlp on host) during attention
        nc.scalar.dma_start(
            wout_sb, io["wout"].ap().rearrange("(dc p) c -> p dc c", p=P)
        )
        nc.scalar.dma_start(w2sb, io["w2"].ap().rearrange("(o p) c -> p o c", p=P))
        ones_t = pb.tile([P, 64], f16, tag="ones", name="ones_t", bufs=1)
        nc.vector.memset(ones_t, 1.0)

        # One flat software pipeline over all (head-pair, q-chunk, k-tile)
        # steps: scores lead attnv by SKEW steps ACROSS chunk boundaries, so
        # the ScalarE exp stream never drains while the PE finishes a
        # chunk's last attnv accumulations.  Each chunk's normalize is
        # emitted NORM_DELAY steps into the next chunk so its rank-1
        # broadcast matmuls wait on nothing by the time the PE reaches them.
        NCH = HP * QH
        SKEW = 2
        NORM_DELAY = 6

        class Chunk:
            def __init__(self, ci):
                self.hp, self.qh = divmod(ci, QH)
                self.q0 = self.qh * QW
                self.probs = [None] * ST_F
                self.psUA = ps_u.tile([P, QW], f32, tag="attA", name="psUA")
                self.psUB = ps_u.tile([P, QW], f32, tag="attB", name="psUB")

        chunks = {}

        def scores(ci, kt):
            if ci not in chunks:
                chunks[ci] = Chunk(ci)
            c = chunks[ci]
            # A in cols 0:QW (PE rows 0:64), B in cols QW:2QW (rows 64:128);
            # the two MMs run concurrently in the array.
            ps = ps_sc.tile([P, 2 * QW], f32, tag="scoreAB", name="psAB")
            nc.tensor.matmul(
                ps[:, 0:QW], kT[0:HD, c.hp, kt * P : (kt + 1) * P],
                qT[0:HD, c.hp, c.q0 : c.q0 + QW], start=True, stop=True,
            )
            nc.tensor.matmul(
                ps[:, QW : 2 * QW], kT[HD:P, c.hp, kt * P : (kt + 1) * P],
                qT[HD:P, c.hp, c.q0 : c.q0 + QW], start=True, stop=True,
            )
            pr = pb.tile([P, 2 * QW], bf16, tag="probs", name="probs", bufs=4)
            nc.scalar.activation(out=pr, in_=ps, func=AF.Exp, scale=0.125)
            c.probs[kt] = pr

        def attnv(ci, kt):
            c = chunks[ci]
            nc.tensor.matmul(
                c.psUA[0:65, :], v_aug[:, kt, 2 * c.hp, :],
                c.probs[kt][:, 0:QW],
                start=(kt == 0), stop=(kt == ST_F - 1),
            )
            nc.tensor.matmul(
                c.psUB[0:65, :], v_aug[:, kt, 2 * c.hp + 1, :],
                c.probs[kt][:, QW : 2 * QW],
                start=(kt == 0), stop=(kt == ST_F - 1),
            )
            c.probs[kt] = None

        def normalize(ci):
            c = chunks.pop(ci)
            # 1/denominator broadcast across 64 partitions via rank-1 mm.
            # approx reciprocal (18 bits) + fp16 rank-1 matmul: the exact
            # single-partition reciprocal costs 3.3us and an fp32 matmul
            # runs in the 4-pass mode — both stalled the PE FIFO.
            rec32 = pbs.tile([P, 2 * QW], f32, tag="rec32", name="rec32")
            nc.vector.reciprocal(rec32[64:65, 0:QW], c.psUA[64:65, :])
            nc.vector.reciprocal(rec32[64:65, QW : 2 * QW], c.psUB[64:65, :])
            rec = pbs.tile([P, 2 * QW], f16, tag="rec", name="rec")
            nc.vector.tensor_copy(rec[64:65, :], rec32[64:65, :])
            bc = ps_sc.tile([P, 2 * QW], f32, tag="scoreAB", name="bcAB")
            nc.tensor.matmul(bc[0:64, 0:QW], ones_t[64:65, :],
                             rec[64:65, 0:QW], start=True, stop=True)
            nc.tensor.matmul(bc[0:64, QW : 2 * QW], ones_t[64:65, :],
                             rec[64:65, QW : 2 * QW], start=True, stop=True)
            bcs = pbs.tile([P, 2 * QW], f32, tag="bcs", name="bcs")
            nc.vector.tensor_copy(bcs[0:64, :], bc[0:64, :])
            nc.vector.tensor_mul(attnT[0:64, c.hp, c.q0 : c.q0 + QW],
                                 c.psUA[0:64, :], bcs[0:64, 0:QW])
            nc.vector.tensor_mul(attnT[64:128, c.hp, c.q0 : c.q0 + QW],
                                 c.psUB[0:64, :], bcs[0:64, QW : 2 * QW])

        n_steps = NCH * ST_F
        for i in range(n_steps + SKEW):
            if i < n_steps:
                scores(i // ST_F, i % ST_F)
            j = i - SKEW
            if j >= 0:
                attnv(j // ST_F, j % ST_F)
            jn = j - NORM_DELAY  # chunk ci done when its attnv 15 was at step
            if jn >= 0 and jn % ST_F == ST_F - 1:
                normalize(jn // ST_F)
        normalize(NCH - 1)
    free_vaug()
    free_kT()
    free_qT()

    # ======== phase C: out-proj + residual + LN2, fused per s-tile ========
    x_mid, free_xmid = big([P, ST_O, D], f32, "x_mid", side="right")
    y2T, free_y2T = big([P, DC, SO], bf16, "y2T", side="right")

    with tc.tile_pool(name="pc", bufs=3) as pc, \
         tc.tile_pool(name="pcs", bufs=4) as pcs, \
         tc.tile_pool(name="psc", bufs=2, space="PSUM") as psc:
        g1_t = bcast_row(pc, "g1")
        a2_t = bcast_row(pc, "a2", dt=bf16)
        c2_t = bcast_row(pc, "c2", dt=bf16)
        for st in range(ST_O):
            x_t = pc.tile([P, D], f32, tag="xin4", name="x_t4")
            nc.sync.dma_start(x_t, io["x_own"].ap()[st * P : (st + 1) * P, :])
            ps = psc.tile([P, D], f32, tag="outproj", name="op_ps")
            for dc in range(DC):
                mmw(
                    ps,
                    attnT[:, dc, st * P : (st + 1) * P],
                    wout_sb[:, dc, :],
                    start=(dc == 0),
                    stop=(dc == DC - 1),
                )
            t = pc.tile([P, D], f32, tag="op_t", name="op_t")
            nc.vector.tensor_mul(t, ps, g1_t)
            nc.gpsimd.tensor_add(x_mid[:, st, :], t, x_t)
            y2 = pc.tile([P, D], bf16, tag="y2", name="y2")
            layernorm_mod(pcs, x_mid[:, st, :], y2, a2_t, c2_t)
            nc.sync.dma_start(
                out=y2T[:, :, st * P : (st + 1) * P], in_=y2, transpose=True
            )
    free_wout()
    free_attnT()

    # ================ phase D: fc1 + gelu ================
    hT, free_hT = big([P, JT, SO], bf16, "hT")
    with tc.tile_pool(name="pd", bufs=3) as pd, \
         tc.tile_pool(name="psd", bufs=2, space="PSUM") as psd:
        fb1_t = pd.tile([P, JT], f32, tag="fb1", name="fb1_t", bufs=1)
        nc.sync.dma_start(fb1_t, io["fb1"].ap().rearrange("(o p) -> p o", p=P))
        # w1 host-pre-arranged to [jt, p, dc*128]: 2KB contiguous per partition
        w1_r = io["w1"].ap().rearrange("(jt p) x -> jt p x", p=P)
        for jt in range(JT):
            w1_t = pd.tile([P, DC, P], bf16, tag="w1", name="w1_t")
            nc.scalar.dma_start(
                w1_t, w1_r[jt].rearrange("p (dc c) -> p dc c", dc=DC)
            )
            # q-chunked so the first chunks only need half of y2T (st 0..3):
            # fc1 overlaps phase C's LN2/transpose tail.
            for qh in range(SO // NB):
                ps = psd.tile([P, NB], f32, tag="fc1", name="fc1_ps", bufs=4)
                for dc in range(DC):
                    nc.tensor.matmul(
                        ps,
                        w1_t[:, dc, :],
                        y2T[:, dc, qh * NB : (qh + 1) * NB],
                        start=(dc == 0),
                        stop=(dc == DC - 1),
                    )
                hdst = hT[:, jt, qh * NB : (qh + 1) * NB]
                if os.environ.get("KERNEL_SIM_GELU"):
                    # sim lacks Gelu tables: tanh-approx gelu from primitives
                    u = pd.tile([P, NB], f32, tag="gelu_u", name="gelu_u")
                    nc.vector.tensor_scalar_add(u, ps, fb1_t[:, jt : jt + 1])
                    t = pd.tile([P, NB], f32, tag="gelu_t", name="gelu_t")
                    nc.vector.tensor_mul(t, u, u)
                    nc.vector.tensor_mul(t, t, u)
                    nc.vector.scalar_tensor_tensor(
                        out=t, in0=t, scalar=0.044715, in1=u,
                        op0=OP.mult, op1=OP.add,
                    )
                    nc.scalar.activation(
                        out=t, in_=t, func=AF.Tanh, scale=0.7978845608028654
                    )
                    nc.vector.tensor_scalar(
                        out=t, in0=t, scalar1=1.0, scalar2=0.5,
                        op0=OP.add, op1=OP.mult,
                    )
                    nc.vector.tensor_mul(hdst, u, t)
                else:
                    nc.scalar.activation(
                        out=hdst,
                        in_=ps,
                        func=AF.Gelu_apprx_tanh,
                        bias=fb1_t[:, jt : jt + 1],
                    )
    free_y2T()

    # ======== phase E: fc2 (w2 pre-scaled by gate) + final residual ========
    with tc.tile_pool(name="pe", bufs=3) as pe, \
         tc.tile_pool(name="pse", bufs=2, space="PSUM") as pse:
        gb2_t = bcast_row(pe, "gb2")
        for st in range(ST_O):
            ps = pse.tile([P, D], f32, tag="fc2", name="fc2_ps")
            for jt in range(JT):
                mmw(
                    ps,
                    hT[:, jt, st * P : (st + 1) * P],
                    w2sb[:, jt, :],
                    start=(jt == 0),
                    stop=(jt == JT - 1),
                )
            t = pe.tile([P, D], f32, tag="fin_t", name="fin_t")
            nc.vector.scalar_tensor_tensor(
                out=t, in0=ps, scalar=1.0, in1=x_mid[:, st, :],
                op0=OP.mult, op1=OP.add,
            )
            o_t = pe.tile([P, D], f32, tag="out", name="o_t")
            nc.gpsimd.tensor_add(o_t, t, gb2_t)
            nc.sync.dma_start(io["out"].ap()[st * P : (st + 1) * P, :], o_t)
    free_hT()
    free_w2()
    free_xmid()


def build_nc():
    import concourse.tile as tile
    import concourse.mybir as mybir
    from concourse import bacc

    f32 = mybir.dt.float32
    bf16 = mybir.dt.bfloat16

    nc = bacc.Bacc("TRN2", target_bir_lowering=False, debug=False)
    io = {}
    io["x_own"] = nc.dram_tensor("x_own", [SO, D], f32, kind="ExternalInput")
    io["x_oth"] = nc.dram_tensor("x_oth", [SO, D], f32, kind="ExternalInput")
    io["cos"] = nc.dram_tensor("cos", [P, ST_F * HD], bf16, kind="ExternalInput")
    io["sin"] = nc.dram_tensor("sin", [P, ST_F * HD], bf16, kind="ExternalInput")
    io["wqkv"] = nc.dram_tensor("wqkv", [D, 3 * D], bf16, kind="ExternalInput")
    io["wout"] = nc.dram_tensor("wout", [D, D], bf16, kind="ExternalInput")
    io["w1"] = nc.dram_tensor("w1", [JT * P, DC * P], bf16, kind="ExternalInput")
    io["w2"] = nc.dram_tensor("w2", [J, D], bf16, kind="ExternalInput")
    for name in ["a1", "c1", "a2", "c2"]:
        io[name] = nc.dram_tensor(name, [D], bf16, kind="ExternalInput")
    for name in ["g1", "gb2"]:
        io[name] = nc.dram_tensor(name, [D], f32, kind="ExternalInput")
    io["fb1"] = nc.dram_tensor("fb1", [J], f32, kind="ExternalInput")
    io["out"] = nc.dram_tensor("out", [SO, D], f32, kind="ExternalOutput")

    with tile.TileContext(nc) as tc:
        with ExitStack() as ctx:
            _emit(ctx, nc, tc, io)
    nc.finalize()
    return nc


def host_prep(inputs):
    """Build the 8 per-core input maps from the full problem inputs."""
    import ml_dtypes

    bf = ml_dtypes.bfloat16
    x = np.asarray(inputs["x"], np.float32)
    sigma_emb = np.asarray(inputs["sigma_emb"], np.float32)
    ada = sigma_emb @ np.asarray(inputs["ada_W"], np.float32) + np.asarray(
        inputs["ada_b"], np.float32
    )
    ada = ada.reshape(B, 6, D)
    shift_msa, scale_msa, gate_msa, shift_mlp, scale_mlp, gate_mlp = (
        ada[:, i] for i in range(6)
    )
    ln1_s = np.asarray(inputs["ln1_scale"], np.float32)
    ln1_b = np.asarray(inputs["ln1_bias"], np.float32)
    ln2_s = np.asarray(inputs["ln2_scale"], np.float32)
    ln2_b = np.asarray(inputs["ln2_bias"], np.float32)

    a1 = (ln1_s[None] * (1.0 + scale_msa)).astype(bf)  # [B, D]
    c1 = (ln1_b[None] * (1.0 + scale_msa) + shift_msa).astype(bf)
    a2 = (ln2_s[None] * (1.0 + scale_mlp)).astype(bf)
    c2 = (ln2_b[None] * (1.0 + scale_mlp) + shift_mlp).astype(bf)
    gb2 = gate_mlp * np.asarray(inputs["fc2_b"], np.float32)[None]

    # rope tables (match reference)
    inv_freq = 1.0 / (10000.0 ** (np.arange(0, HD, 2, dtype=np.float32) / HD))
    t = np.arange(S, dtype=np.float32)
    freqs = np.einsum("n,d->nd", t, inv_freq)
    emb = np.concatenate([freqs, freqs], axis=-1)  # [S, HD]
    cos = np.cos(emb).astype(bf)
    sin_signed = np.sin(emb).astype(np.float32)
    sin_signed[:, : HD // 2] *= -1.0  # fold rotate_half sign
    sin_signed = sin_signed.astype(bf)

    wqkv = np.asarray(inputs["W_qkv"], np.float32).astype(bf)
    wout = np.asarray(inputs["W_out"], np.float32).astype(bf)
    # [D, J] -> [jt, p, dc, c] with w1[dc*128+p, jt*128+c] at [jt, p, dc, c]
    w1 = np.asarray(inputs["fc1_W"], np.float32).astype(bf)
    w1 = np.ascontiguousarray(
        w1.reshape(DC, P, JT, P).transpose(2, 1, 0, 3).reshape(JT * P, DC * P)
    )
    w2f = np.asarray(inputs["fc2_W"], np.float32)
    fb1 = np.asarray(inputs["fc1_b"], np.float32)

    in_maps = []
    for c in range(N_CORES):
        b, h = c // 2, c % 2
        own = slice(h * SO, (h + 1) * SO)
        oth = slice((1 - h) * SO, (2 - h) * SO)
        in_maps.append(
            {
                "x_own": np.ascontiguousarray(x[b, own]),
                "x_oth": np.ascontiguousarray(x[b, oth]),
                # [S, HD] -> [p, st*HD] so each partition line is contiguous
                "cos": np.ascontiguousarray(
                    np.concatenate([cos[own], cos[oth]], 0)
                    .reshape(ST_F, P, HD).transpose(1, 0, 2).reshape(P, ST_F * HD)
                ),
                "sin": np.ascontiguousarray(
                    np.concatenate([sin_signed[own], sin_signed[oth]], 0)
                    .reshape(ST_F, P, HD).transpose(1, 0, 2).reshape(P, ST_F * HD)
                ),
                "wqkv": wqkv,
                "wout": wout,
                "w1": w1,
                "w2": (w2f * gate_mlp[b][None, :]).astype(bf),
                "a1": np.ascontiguousarray(a1[b]),
                "c1": np.ascontiguousarray(c1[b]),
                "g1": np.ascontiguousarray(gate_msa[b]),
                "a2": np.ascontiguousarray(a2[b]),
                "c2": np.ascontiguousarray(c2[b]),
                "gb2": np.ascontiguousarray(gb2[b]),
                "fb1": fb1,
            }
        )
    return in_maps


_NC_CACHE = {}


def kernel(**inputs) -> np.ndarray:
    import sys

    if "/opt/trn_rl_repo" not in sys.path:
        sys.path.insert(0, "/opt/trn_rl_repo")
    from concourse.bass_utils import run_bass_kernel_spmd

    in_maps = host_prep(inputs)
    if "nc" not in _NC_CACHE:
        _NC_CACHE["nc"] = build_nc()
    nc = _NC_CACHE["nc"]
    res = run_bass_kernel_spmd(
        nc,
        in_maps,
        core_ids=list(range(N_CORES)),
        trace=bool(int(os.environ.get("KERNEL_TRACE", "0"))),
    )
    out = np.empty((B, S, D), np.float32)
    for c in range(N_CORES):
        b, h = c // 2, c % 2
        out[b, h * SO : (h + 1) * SO] = res.results[c]["out"]
    _NC_CACHE["last_result"] = res
    return out



# revision 48
# speedup vs baseline: 1.3797x; 1.3797x over previous
"""DDiT block kernel for 8 Trainium2 NeuronCores — v3 (pipelined attention).

Sharding: core c = (b = c//2, half = c%2).  Each core computes the block
output for its (batch, sequence-half): 1024 rows of 2048.  K/V are computed
redundantly for the full sequence on each core.  No collectives.

v3 changes vs v2 (trace-driven):
- Attention rebuilt as a q-chunked (512) software pipeline with
  double-buffered score PSUM: ScalarE streams one Exp per k-tile
  ([128,1024] over both heads of a pair) back-to-back while the PE runs
  2 k-tiles ahead on scores and 1 behind on attn@V.  v2 serialised
  PE->exp->PE per k-tile (bufs=1 score psum), which also kept the PE
  HAM-throttled at half clock for the whole phase (~620us -> target ~300).
- wqkv DMA split per d-tile so the first QKV matmul starts after ~1/8 of
  the weight load; weight streams (wqkv/wout/w2/w1) moved to the second
  HWDGE queue (ScalarE-issued) so they never queue behind x loads and
  SBUF transposes on the sync queue.
- fc1 weights re-laid-out on host to [jt, p, dc*128] so each per-jt DMA
  reads 2KB contiguous per partition (v2's layout produced 256B packets
  at ~74GB/s, gating fc1).
"""

import os
from contextlib import ExitStack

import numpy as np

B, S, D, H = 4, 2048, 1024, 16
HD = D // H  # 64
J = 4 * D  # 4096
SO = S // 2  # 1024 rows per core
SF = S  # full sequence
P = 128
EPS = 1e-6
N_CORES = 8

ST_F = SF // P  # 16 s-tiles full seq
ST_O = SO // P  # 8 s-tiles own half
DC = D // P  # 8 d-tiles
JT = J // P  # 32 j-tiles
HP = H // 2  # 8 head pairs


def _emit(ctx, nc, tc, io):
    import concourse.bass as bass
    import concourse.mybir as mybir

    f32 = mybir.dt.float32
    bf16 = mybir.dt.bfloat16
    f16 = mybir.dt.float16
    f8 = mybir.dt.float8e4
    AF = mybir.ActivationFunctionType
    OP = mybir.AluOpType
    DR = mybir.MatmulPerfMode.DoubleRow

    def big(shape, dtype, name, side="left"):
        t, free = tc.tile(shape, dtype, name=name, side=side)
        return t, free

    def bcast_row(pool, key, n=D, dt=f32, engine=None):
        """DMA a [n] dram row into a [P, n] sbuf tile, replicated across partitions."""
        ap_1d = io[key].ap()
        t = pool.tile([P, n], dt, tag=f"row_{key}", name=f"row_{key}", bufs=1)
        src = bass.AP(
            tensor=ap_1d.tensor,
            offset=ap_1d.offset,
            ap=[[0, P], list(ap_1d.ap[0])],
        )
        (engine or nc.sync).dma_start(out=t, in_=src)
        return t

    NB = 512  # psum bank width (f32 cols); matmuls may not cross banks

    def mmw(ps, lhsT, rhs, start, stop):
        for o in range(0, rhs.shape[-1], NB):
            nc.tensor.matmul(
                ps[:, o : o + NB], lhsT, rhs[..., o : o + NB],
                start=start, stop=stop,
            )

    eps_t, _free_eps = tc.tile([P, 1], f32, name="eps_t")
    nc.vector.memset(eps_t, EPS)

    def layernorm_mod(pool, x_ap, out_bf, a_bf, c_bf):
        # out_bf16 = ((x - mean) * rstd) * A + C   (A, C bf16 rows)
        stats = pool.tile([P, 2, 6], f32, tag="bnstats", name="stats")
        mv = pool.tile([P, 2], f32, tag="bnaggr", name="mv")
        xv = x_ap.rearrange("p (g k) -> p g k", g=2)
        for g in range(2):
            nc.vector.bn_stats(out=stats[:, g, :], in_=xv[:, g, :])
        nc.vector.bn_aggr(out=mv, in_=stats)
        rstd = pool.tile([P, 1], f32, tag="rstd", name="rstd")
        nc.scalar.activation(out=rstd, in_=mv[:, 1:2], func=AF.Sqrt, bias=eps_t)
        nc.vector.reciprocal(out=rstd, in_=rstd)
        tmp = pool.tile([P, D], bf16, tag="lntmp", name="lntmp")
        nc.vector.tensor_scalar(
            out=tmp, in0=x_ap, scalar1=mv[:, 0:1], scalar2=rstd,
            op0=OP.subtract, op1=OP.mult,
        )
        tmp2 = pool.tile([P, D], bf16, tag="lntmp2", name="lntmp2")
        nc.vector.tensor_mul(tmp2, tmp, a_bf)
        nc.vector.tensor_add(out_bf, tmp2, c_bf)

    # ======== persistent SBUF tensors (phase A) ========
    wqkv_sb, free_wqkv = big([P, DC, 3 * D], bf16, "wqkv_sb")  # 48KB/p
    yT, free_yT = big([P, DC, 2 * P], bf16, "yT")  # rolling 2-tile buffer
    qT, free_qT = big([P, HP, SO], bf16, "qT", side="right")
    kT, free_kT = big([P, HP, SF], bf16, "kT", side="right")
    v_aug, free_vaug = big([P, ST_F, H, 65], bf16, "v_aug", side="right")

    # ================ phase A: LN1 + QKV + rope, fused per s-tile ========
    with tc.tile_pool(name="pa", bufs=3) as pa, \
         tc.tile_pool(name="pas", bufs=4) as pas, \
         tc.tile_pool(name="psa", bufs=2, space="PSUM") as psa:
        # Warm the ACT sqrt/exp table set before the weight-DMA issue
        # instructions claim the ScalarE queue — the first LN rstd otherwise
        # waits ~14us for its PSEUDO_LOAD_ACT_FUNC_SET.
        warm_t = pa.tile([P, 1], f32, tag="warm", name="warm_t", bufs=1)
        nc.scalar.activation(out=warm_t, in_=eps_t, func=AF.Sqrt, bias=eps_t)
        # Small constants first on the scalar HWDGE queue; the wqkv chunk
        # issues are spread across the first loop iterations so the LN
        # rstd/modulate ACT work is never queued behind them.  The sync
        # queue starts with the x tiles so LN begins ASAP.
        a1_t = bcast_row(pa, "a1", dt=bf16, engine=nc.scalar)
        c1_t = bcast_row(pa, "c1", dt=bf16, engine=nc.scalar)
        cos_t = pa.tile([P, ST_F, HD], bf16, tag="cos", name="cos_t", bufs=1)
        sin_t = pa.tile([P, ST_F, HD], bf16, tag="sin", name="sin_t", bufs=1)
        # host pre-arranged to [p, t*d]: 2KB contiguous per partition
        nc.scalar.dma_start(cos_t, io["cos"].ap().rearrange("p (t d) -> p t d", d=HD))
        nc.scalar.dma_start(sin_t, io["sin"].ap().rearrange("p (t d) -> p t d", d=HD))
        wqkv_r = io["wqkv"].ap().rearrange("(dc p) c -> p dc c", p=P)
        wqkv_next = [0]

        def issue_wqkv(upto):
            while wqkv_next[0] < min(upto, DC):
                dc = wqkv_next[0]
                nc.scalar.dma_start(wqkv_sb[:, dc, :], wqkv_r[:, dc, :])
                wqkv_next[0] += 1

        issue_wqkv(2)
        nc.vector.memset(v_aug[:, :, :, 64:65], 1.0)

        def rope(out_bf, qn, st):
            """Rotary on bf16 [P, 1024] (16 head-halves of 64)."""
            qv = qn.rearrange("p (h d) -> p h d", h=16)
            cos_b = cos_t[:, st, None, :].to_broadcast((P, 16, HD))
            sin_b = sin_t[:, st, None, :].to_broadcast((P, 16, HD))
            t1 = pas.tile([P, 16, HD], bf16, tag="ropet1", name="t1")
            nc.vector.tensor_mul(t1, qv, cos_b)
            qshuf = qn.rearrange("p (h two j) -> p h two j", h=16, two=2)[:, :, ::-1, :]
            t2 = pas.tile([P, 16, HD], bf16, tag="ropet2", name="t2")
            nc.vector.tensor_mul(
                t2.rearrange("p h (two j) -> p h two j", two=2),
                qshuf,
                sin_b.rearrange("p h (two j) -> p h two j", two=2),
            )
            nc.vector.tensor_add(out_bf.rearrange("p (h d) -> p h d", h=16), t1, t2)

        for st in range(ST_F):
            src = io["x_own"] if st < ST_O else io["x_oth"]
            row0 = (st % ST_O) * P
            x_t = pa.tile([P, D], f32, tag="xin", name="x_t")
            nc.sync.dma_start(x_t, src.ap()[row0 : row0 + P, :])
            y_t = pa.tile([P, D], bf16, tag="y_nat", name="y_t")
            layernorm_mod(pas, x_t, y_t, a1_t, c1_t)
            # remaining wqkv chunks AFTER st0's LN ACT ops but BEFORE the
            # first matmuls that read them (reads must follow the DMA in
            # trace order or Tile cannot create the dependency)
            issue_wqkv(DC)
            sl = (st % 2) * P
            nc.sync.dma_start(
                out=yT[:, :, sl : sl + P], in_=y_t, transpose=True
            )

            blocks = []
            if st < ST_O:
                blocks.append(("q", 0))
            blocks.append(("k", D))
            blocks.append(("v", 2 * D))
            for kind, c0 in blocks:
                ps = psa.tile([P, D], f32, tag="qkv_ps", name="qkv_ps")
                for dc in range(DC):
                    mmw(
                        ps,
                        yT[:, dc, sl : sl + P],
                        wqkv_sb[:, dc, c0 : c0 + D],
                        start=(dc == 0),
                        stop=(dc == DC - 1),
                    )
                if kind == "v":
                    nc.scalar.copy(
                        out=v_aug[:, st, :, 0:64],
                        in_=ps.rearrange("p (h d) -> p h d", h=16),
                    )
                else:
                    qn = pa.tile([P, D], bf16, tag="qn", name="qn")
                    nc.scalar.copy(out=qn, in_=ps)
                    rot = pa.tile([P, D], bf16, tag="rot", name="rot")
                    rope(rot, qn, st)
                    dst = qT if kind == "q" else kT
                    nc.sync.dma_start(
                        out=dst[:, :, st * P : (st + 1) * P], in_=rot,
                        transpose=True,
                    )
    free_yT()
    free_wqkv()

    # ======== persistent SBUF tensors (phase B+) ========
    # fp8 weights are host-packed in DoubleRow pair layout [.., k2, 2, ..]
    # and pre-scaled (w1/wout x16, w2 x64) to clear the e4m3 subnormal zone;
    # the scales are compensated in each epilogue.
    w2sb, free_w2 = big([P, JT // 2, 2, D], f8, "w2sb")  # 32KB/p
    attnT, free_attnT = big([P, DC, SO], bf16, "attnT")
    wout_sb, free_wout = big([P, DC, D], bf16, "wout_sb")

    # ======== phase B: attention (q-chunked, ScalarE-saturated pipeline) ====
    QW = 512  # q-chunk width; scores for both heads of a pair share one exp
    QH = SO // QW
    with tc.tile_pool(name="pb", bufs=4) as pb, \
         tc.tile_pool(name="pbs", bufs=2) as pbs, \
         tc.tile_pool(name="ps_sc", bufs=2, space="PSUM") as ps_sc, \
         tc.tile_pool(name="ps_u", bufs=2, space="PSUM") as ps_u:
        # prefetch wout + w2 (pre-scaled by gate_mlp on host) during attention
        nc.scalar.dma_start(
            wout_sb, io["wout"].ap().rearrange("(dc p) c -> p dc c", p=P)
        )
        nc.scalar.dma_start(
            w2sb, io["w2"].ap().rearrange("p (j o c) -> p j o c", j=JT // 2, o=2)
        )
        ones_t = pb.tile([P, 64], f16, tag="ones", name="ones_t", bufs=1)
        nc.vector.memset(ones_t, 1.0)

        # One flat software pipeline over all (head-pair, q-chunk, k-tile)
        # steps: scores lead attnv by SKEW steps ACROSS chunk boundaries, so
        # the ScalarE exp stream never drains while the PE finishes a
        # chunk's last attnv accumulations.  Each chunk's normalize is
        # emitted NORM_DELAY steps into the next chunk so its rank-1
        # broadcast matmuls wait on nothing by the time the PE reaches them.
        NCH = HP * QH
        SKEW = 2
        NORM_DELAY = 6

        class Chunk:
            def __init__(self, ci):
                self.hp, self.qh = divmod(ci, QH)
                self.q0 = self.qh * QW
                self.probs = [None] * ST_F
                self.psUA = ps_u.tile([P, QW], f32, tag="attA", name="psUA")
                self.psUB = ps_u.tile([P, QW], f32, tag="attB", name="psUB")

        chunks = {}

        def scores(ci, kt):
            if ci not in chunks:
                chunks[ci] = Chunk(ci)
            c = chunks[ci]
            # A in cols 0:QW (PE rows 0:64), B in cols QW:2QW (rows 64:128);
            # the two MMs run concurrently in the array.
            ps = ps_sc.tile([P, 2 * QW], f32, tag="scoreAB", name="psAB")
            nc.tensor.matmul(
                ps[:, 0:QW], kT[0:HD, c.hp, kt * P : (kt + 1) * P],
                qT[0:HD, c.hp, c.q0 : c.q0 + QW], start=True, stop=True,
            )
            nc.tensor.matmul(
                ps[:, QW : 2 * QW], kT[HD:P, c.hp, kt * P : (kt + 1) * P],
                qT[HD:P, c.hp, c.q0 : c.q0 + QW], start=True, stop=True,
            )
            pr = pb.tile([P, 2 * QW], bf16, tag="probs", name="probs", bufs=4)
            nc.scalar.activation(out=pr, in_=ps, func=AF.Exp, scale=0.125)
            c.probs[kt] = pr

        def attnv(ci, kt):
            c = chunks[ci]
            nc.tensor.matmul(
                c.psUA[0:65, :], v_aug[:, kt, 2 * c.hp, :],
                c.probs[kt][:, 0:QW],
                start=(kt == 0), stop=(kt == ST_F - 1),
            )
            nc.tensor.matmul(
                c.psUB[0:65, :], v_aug[:, kt, 2 * c.hp + 1, :],
                c.probs[kt][:, QW : 2 * QW],
                start=(kt == 0), stop=(kt == ST_F - 1),
            )
            c.probs[kt] = None

        def normalize(ci):
            c = chunks.pop(ci)
            # 1/denominator broadcast across 64 partitions via rank-1 mm.
            # approx reciprocal (18 bits) + fp16 rank-1 matmul: the exact
            # single-partition reciprocal costs 3.3us and an fp32 matmul
            # runs in the 4-pass mode — both stalled the PE FIFO.
            rec32 = pbs.tile([P, 2 * QW], f32, tag="rec32", name="rec32")
            nc.vector.reciprocal(rec32[64:65, 0:QW], c.psUA[64:65, :])
            nc.vector.reciprocal(rec32[64:65, QW : 2 * QW], c.psUB[64:65, :])
            rec = pbs.tile([P, 2 * QW], f16, tag="rec", name="rec")
            nc.vector.tensor_copy(rec[64:65, :], rec32[64:65, :])
            bc = ps_sc.tile([P, 2 * QW], f32, tag="scoreAB", name="bcAB")
            nc.tensor.matmul(bc[0:64, 0:QW], ones_t[64:65, :],
                             rec[64:65, 0:QW], start=True, stop=True)
            nc.tensor.matmul(bc[0:64, QW : 2 * QW], ones_t[64:65, :],
                             rec[64:65, QW : 2 * QW], start=True, stop=True)
            bcs = pbs.tile([P, 2 * QW], f32, tag="bcs", name="bcs")
            nc.vector.tensor_copy(bcs[0:64, :], bc[0:64, :])
            nc.vector.tensor_mul(attnT[0:64, c.hp, c.q0 : c.q0 + QW],
                                 c.psUA[0:64, :], bcs[0:64, 0:QW])
            nc.vector.tensor_mul(attnT[64:128, c.hp, c.q0 : c.q0 + QW],
                                 c.psUB[0:64, :], bcs[0:64, QW : 2 * QW])

        n_steps = NCH * ST_F
        for i in range(n_steps + SKEW):
            if i < n_steps:
                scores(i // ST_F, i % ST_F)
            j = i - SKEW
            if j >= 0:
                attnv(j // ST_F, j % ST_F)
            jn = j - NORM_DELAY  # chunk ci done when its attnv 15 was at step
            if jn >= 0 and jn % ST_F == ST_F - 1:
                normalize(jn // ST_F)
        normalize(NCH - 1)
    free_vaug()
    free_kT()
    free_qT()

    # ======== phase C: out-proj + residual + LN2, fused per s-tile ========
    x_mid, free_xmid = big([P, ST_O, D], f32, "x_mid", side="right")
    y2T, free_y2T = big([P, DC, SO], bf16, "y2T", side="right")
    y2T8, free_y2T8 = big([P, DC, SO], f8, "y2T8", side="right")

    with tc.tile_pool(name="pc", bufs=3) as pc, \
         tc.tile_pool(name="pcs", bufs=4) as pcs, \
         tc.tile_pool(name="psc", bufs=2, space="PSUM") as psc:
        g1_t = bcast_row(pc, "g1")
        a2_t = bcast_row(pc, "a2", dt=bf16)
        c2_t = bcast_row(pc, "c2", dt=bf16)
        for st in range(ST_O):
            x_t = pc.tile([P, D], f32, tag="xin4", name="x_t4")
            nc.sync.dma_start(x_t, io["x_own"].ap()[st * P : (st + 1) * P, :])
            ps = psc.tile([P, D], f32, tag="outproj", name="op_ps")
            for dc in range(DC):
                mmw(
                    ps,
                    attnT[:, dc, st * P : (st + 1) * P],
                    wout_sb[:, dc, :],
                    start=(dc == 0),
                    stop=(dc == DC - 1),
                )
            t = pc.tile([P, D], f32, tag="op_t", name="op_t")
            nc.vector.tensor_mul(t, ps, g1_t)
            nc.gpsimd.tensor_add(x_mid[:, st, :], t, x_t)
            y2 = pc.tile([P, D], bf16, tag="y2", name="y2")
            layernorm_mod(pcs, x_mid[:, st, :], y2, a2_t, c2_t)
            nc.sync.dma_start(
                out=y2T[:, :, st * P : (st + 1) * P], in_=y2, transpose=True
            )
            nc.vector.tensor_copy(
                y2T8[:, :, st * P : (st + 1) * P],
                y2T[:, :, st * P : (st + 1) * P],
            )
    free_wout()
    free_attnT()

    # ================ phase D: fc1 + gelu ================
    hT, free_hT = big([P, JT, SO], f8, "hT")
    with tc.tile_pool(name="pd", bufs=3) as pd, \
         tc.tile_pool(name="psd", bufs=2, space="PSUM") as psd:
        fb1_t = pd.tile([P, JT], f32, tag="fb1", name="fb1_t", bufs=1)
        nc.sync.dma_start(fb1_t, io["fb1"].ap().rearrange("(o p) -> p o", p=P))
        # w1 host-pre-arranged to [jt, p, dc2*2*128] DR pairs, 2KB lines
        w1_r = io["w1"].ap().rearrange("(jt p) x -> jt p x", p=P)
        for jt in range(JT):
            w1_t = pd.tile([P, DC // 2, 2, P], f8, tag="w1", name="w1_t")
            nc.scalar.dma_start(
                w1_t, w1_r[jt].rearrange("p (a o c) -> p a o c", a=DC // 2, o=2)
            )
            # q-chunked so the first chunks only need half of y2T (st 0..3):
            # fc1 overlaps phase C's LN2/transpose tail.
            for qh in range(SO // NB):
                ps = psd.tile([P, NB], f32, tag="fc1", name="fc1_ps", bufs=4)
                for dc2 in range(DC // 2):
                    nc.tensor.matmul(
                        ps,
                        w1_t[:, dc2, :, :],
                        y2T8[:, 2 * dc2 : 2 * dc2 + 2, qh * NB : (qh + 1) * NB],
                        start=(dc2 == 0),
                        stop=(dc2 == DC // 2 - 1),
                        perf_mode=DR,
                    )
                hdst = hT[:, jt, qh * NB : (qh + 1) * NB]
                if os.environ.get("KERNEL_SIM_GELU"):
                    # sim lacks Gelu tables: tanh-approx gelu from primitives
                    u = pd.tile([P, NB], f32, tag="gelu_u", name="gelu_u")
                    nc.vector.tensor_scalar(
                        out=u, in0=ps, scalar1=1.0 / 16.0,
                        scalar2=fb1_t[:, jt : jt + 1],
                        op0=OP.mult, op1=OP.add,
                    )
                    t = pd.tile([P, NB], f32, tag="gelu_t", name="gelu_t")
                    nc.vector.tensor_mul(t, u, u)
                    nc.vector.tensor_mul(t, t, u)
                    nc.vector.scalar_tensor_tensor(
                        out=t, in0=t, scalar=0.044715, in1=u,
                        op0=OP.mult, op1=OP.add,
                    )
                    nc.scalar.activation(
                        out=t, in_=t, func=AF.Tanh, scale=0.7978845608028654
                    )
                    nc.vector.tensor_scalar(
                        out=t, in0=t, scalar1=1.0, scalar2=0.5,
                        op0=OP.add, op1=OP.mult,
                    )
                    nc.vector.tensor_mul(hdst, u, t)
                else:
                    # scale compensates the x16 host pre-scaling of w1
                    nc.scalar.activation(
                        out=hdst,
                        in_=ps,
                        func=AF.Gelu_apprx_tanh,
                        bias=fb1_t[:, jt : jt + 1],
                        scale=1.0 / 16.0,
                    )
    free_y2T8()
    free_y2T()

    # ======== phase E: fc2 (w2 pre-scaled by gate) + final residual ========
    with tc.tile_pool(name="pe", bufs=3) as pe, \
         tc.tile_pool(name="pse", bufs=2, space="PSUM") as pse:
        gb2_t = bcast_row(pe, "gb2")
        for st in range(ST_O):
            ps = pse.tile([P, D], f32, tag="fc2", name="fc2_ps")
            for jt2 in range(JT // 2):
                for o in range(0, D, NB):
                    nc.tensor.matmul(
                        ps[:, o : o + NB],
                        hT[:, 2 * jt2 : 2 * jt2 + 2, st * P : (st + 1) * P],
                        w2sb[:, jt2, :, o : o + NB],
                        start=(jt2 == 0), stop=(jt2 == JT // 2 - 1),
                        perf_mode=DR,
                    )
            t = pe.tile([P, D], f32, tag="fin_t", name="fin_t")
            # 1/64 compensates the x64 host pre-scaling of w2
            nc.vector.scalar_tensor_tensor(
                out=t, in0=ps, scalar=1.0 / 64.0, in1=x_mid[:, st, :],
                op0=OP.mult, op1=OP.add,
            )
            o_t = pe.tile([P, D], f32, tag="out", name="o_t")
            nc.gpsimd.tensor_add(o_t, t, gb2_t)
            nc.sync.dma_start(io["out"].ap()[st * P : (st + 1) * P, :], o_t)
    free_hT()
    free_w2()
    free_xmid()


def build_nc():
    import concourse.tile as tile
    import concourse.mybir as mybir
    from concourse import bacc

    f32 = mybir.dt.float32
    bf16 = mybir.dt.bfloat16
    f8 = mybir.dt.float8e4

    nc = bacc.Bacc("TRN2", target_bir_lowering=False, debug=False)
    io = {}
    io["x_own"] = nc.dram_tensor("x_own", [SO, D], f32, kind="ExternalInput")
    io["x_oth"] = nc.dram_tensor("x_oth", [SO, D], f32, kind="ExternalInput")
    io["cos"] = nc.dram_tensor("cos", [P, ST_F * HD], bf16, kind="ExternalInput")
    io["sin"] = nc.dram_tensor("sin", [P, ST_F * HD], bf16, kind="ExternalInput")
    io["wqkv"] = nc.dram_tensor("wqkv", [D, 3 * D], bf16, kind="ExternalInput")
    io["wout"] = nc.dram_tensor("wout", [D, D], bf16, kind="ExternalInput")
    io["w1"] = nc.dram_tensor("w1", [JT * P, DC * P], f8, kind="ExternalInput")
    io["w2"] = nc.dram_tensor("w2", [P, JT * D], f8, kind="ExternalInput")
    for name in ["a1", "c1", "a2", "c2"]:
        io[name] = nc.dram_tensor(name, [D], bf16, kind="ExternalInput")
    for name in ["g1", "gb2"]:
        io[name] = nc.dram_tensor(name, [D], f32, kind="ExternalInput")
    io["fb1"] = nc.dram_tensor("fb1", [J], f32, kind="ExternalInput")
    io["out"] = nc.dram_tensor("out", [SO, D], f32, kind="ExternalOutput")

    with tile.TileContext(nc) as tc:
        with ExitStack() as ctx:
            _emit(ctx, nc, tc, io)
    nc.finalize()
    return nc


def host_prep(inputs):
    """Build the 8 per-core input maps from the full problem inputs."""
    import ml_dtypes

    bf = ml_dtypes.bfloat16
    x = np.asarray(inputs["x"], np.float32)
    sigma_emb = np.asarray(inputs["sigma_emb"], np.float32)
    ada = sigma_emb @ np.asarray(inputs["ada_W"], np.float32) + np.asarray(
        inputs["ada_b"], np.float32
    )
    ada = ada.reshape(B, 6, D)
    shift_msa, scale_msa, gate_msa, shift_mlp, scale_mlp, gate_mlp = (
        ada[:, i] for i in range(6)
    )
    ln1_s = np.asarray(inputs["ln1_scale"], np.float32)
    ln1_b = np.asarray(inputs["ln1_bias"], np.float32)
    ln2_s = np.asarray(inputs["ln2_scale"], np.float32)
    ln2_b = np.asarray(inputs["ln2_bias"], np.float32)

    a1 = (ln1_s[None] * (1.0 + scale_msa)).astype(bf)  # [B, D]
    c1 = (ln1_b[None] * (1.0 + scale_msa) + shift_msa).astype(bf)
    a2 = (ln2_s[None] * (1.0 + scale_mlp)).astype(bf)
    c2 = (ln2_b[None] * (1.0 + scale_mlp) + shift_mlp).astype(bf)
    gb2 = gate_mlp * np.asarray(inputs["fc2_b"], np.float32)[None]

    # rope tables (match reference)
    inv_freq = 1.0 / (10000.0 ** (np.arange(0, HD, 2, dtype=np.float32) / HD))
    t = np.arange(S, dtype=np.float32)
    freqs = np.einsum("n,d->nd", t, inv_freq)
    emb = np.concatenate([freqs, freqs], axis=-1)  # [S, HD]
    cos = np.cos(emb).astype(bf)
    sin_signed = np.sin(emb).astype(np.float32)
    sin_signed[:, : HD // 2] *= -1.0  # fold rotate_half sign
    sin_signed = sin_signed.astype(bf)

    f8 = ml_dtypes.float8_e4m3
    wqkv = np.asarray(inputs["W_qkv"], np.float32).astype(bf)
    # fp8 DoubleRow pair layouts: contract index d = dc2*256 + o*128 + p.
    # Weights pre-scaled x16 (wout/w1) / x64 (w2) to clear e4m3 subnormals;
    # compensated by g1/=16 (host), gelu scale=1/16 and fc2 epilogue 1/64.
    wout = np.asarray(inputs["W_out"], np.float32).astype(bf)
    w1 = np.asarray(inputs["fc1_W"], np.float32) * 16.0
    # [D, J] -> [jt, p, dc2, o, c] with w1[dc2*256+o*128+p, jt*128+c]
    w18 = np.ascontiguousarray(
        w1.reshape(DC // 2, 2, P, JT, P)
        .transpose(3, 2, 0, 1, 4)
        .reshape(JT * P, DC * P)
    ).astype(f8)
    w2f = np.asarray(inputs["fc2_W"], np.float32)
    fb1 = np.asarray(inputs["fc1_b"], np.float32)

    in_maps = []
    for c in range(N_CORES):
        b, h = c // 2, c % 2
        own = slice(h * SO, (h + 1) * SO)
        oth = slice((1 - h) * SO, (2 - h) * SO)
        in_maps.append(
            {
                "x_own": np.ascontiguousarray(x[b, own]),
                "x_oth": np.ascontiguousarray(x[b, oth]),
                # [S, HD] -> [p, st*HD] so each partition line is contiguous
                "cos": np.ascontiguousarray(
                    np.concatenate([cos[own], cos[oth]], 0)
                    .reshape(ST_F, P, HD).transpose(1, 0, 2).reshape(P, ST_F * HD)
                ),
                "sin": np.ascontiguousarray(
                    np.concatenate([sin_signed[own], sin_signed[oth]], 0)
                    .reshape(ST_F, P, HD).transpose(1, 0, 2).reshape(P, ST_F * HD)
                ),
                "wqkv": wqkv,
                "wout": wout,
                "w1": w18,
                "w2": np.ascontiguousarray(
                    (w2f * gate_mlp[b][None, :] * 64.0)
                    .reshape(JT // 2, 2, P, D)
                    .transpose(2, 0, 1, 3)
                    .reshape(P, JT * D)
                ).astype(f8),
                "a1": np.ascontiguousarray(a1[b]),
                "c1": np.ascontiguousarray(c1[b]),
                "g1": np.ascontiguousarray(gate_msa[b]),
                "a2": np.ascontiguousarray(a2[b]),
                "c2": np.ascontiguousarray(c2[b]),
                "gb2": np.ascontiguousarray(gb2[b]),
                "fb1": fb1,
            }
        )
    return in_maps


_NC_CACHE = {}


def kernel(**inputs) -> np.ndarray:
    import sys

    if "/opt/trn_rl_repo" not in sys.path:
        sys.path.insert(0, "/opt/trn_rl_repo")
    from concourse.bass_utils import run_bass_kernel_spmd

    in_maps = host_prep(inputs)
    if "nc" not in _NC_CACHE:
        _NC_CACHE["nc"] = build_nc()
    nc = _NC_CACHE["nc"]
    res = run_bass_kernel_spmd(
        nc,
        in_maps,
        core_ids=list(range(N_CORES)),
        trace=bool(int(os.environ.get("KERNEL_TRACE", "0"))),
    )
    out = np.empty((B, S, D), np.float32)
    for c in range(N_CORES):
        b, h = c // 2, c % 2
        out[b, h * SO : (h + 1) * SO] = res.results[c]["out"]
    _NC_CACHE["last_result"] = res
    return out

